# revision 1
# baseline (speedup 1.0000x reference)
"""CBAM-loss (LDAM-style margin cross-entropy) Trainium2 kernel.

Contract: kernel(**inputs) takes the FULL unsharded inputs
(x [32768, 1000] f32, targets [32768] int, cls_num_list [1000] f32,
class_difficulty [1000] f32, epoch int) and returns the scalar mean
loss (float32), matching:

    m_list1 = margins(cls_num_list, class_difficulty, epoch)   # [C]
    out = x; out[i, t_i] -= m_list1[t_i]
    loss = -mean_i(log_softmax(out)[i, t_i])

Decomposition: per row i with xt_i = x[i, t_i], m_i = m_list1[t_i],

    S0_i   = sum_j exp(x_ij)                       <- device (O(B*C))
    S_i    = S0_i - exp(xt_i) + exp(xt_i - m_i)    <- host (O(B))
    loss_i = log(S_i) - (xt_i - m_i)               <- host (O(B))

x ~ N(0,1), so exp(x) needs no max-subtraction in f32. The device does
the single O(B*C) pass — stream x once from HBM (the memory roofline),
exp on ScalarE, row-sum on VectorE — and returns per-row sums S0. The
O(B) gathers, margin tables ("__init__" constants) and epilogue stay
on the host.

Sharding: data-parallel, 4096 rows per core across 8 NeuronCores.
Per core: 32 row-tiles of [128 rows x 1000 cols] (512 KB), streamed
through NBUF SBUF slots; ScalarE exps each tile in place, VectorE
reduces it into s0[:, t]; one 16 KB output DMA at the end.

Raw Bass (not Tile): Tile fuses multiple semaphore waits into one
instruction, which overflows the single inline sync-wait slot of the
TRN2 compute-instruction encodings; here every cross-engine wait is a
standalone wait_ge.
"""

import numpy as np

B, C = 32768, 1000
N_CORES = 8
R = B // N_CORES          # 4096 rows per core
P = 128                   # SBUF partitions
NT = R // P               # 32 row-tiles per core
NBUF = 16                 # x row-tile buffer slots in SBUF (4 KB/partition each)
# DMA chunk sizes in row-tiles: small at the ends (fast ramp-in for the
# first activation, short drain after the last byte), 2 MiB in the middle
# (large transfers sustain ~390 GB/s vs ~310 at 512 KB)
CHUNKS = [1, 1] + [2] * 13 + [1, 1]
assert sum(CHUNKS) == NT - 2
# Tail shaping: at stream end the ACT->DVE pipeline still owes the exps
# and reduces of everything that arrived late, so the last bytes must be
# small AND the pieces before them mostly pre-processed. Tile 31 goes to
# a dedicated SBUF slot as 4 column quarters — 3 of them streamed EARLY
# (processed mid-stream at engine slack), only the 4th arrives last.
# Tile 30 streams as two column halves just before it. Partial sums land
# in extra s0 columns; the host adds them up.
CQ = C // 4               # 250-column quarter
CH = C // 2               # 500-column half
NT_OUT = NT - 2 + 2 + 4   # 36 output cols: tiles 0..29, halves, quarters

ALPHA, POW_P, BETA = 0.5, 2.0, 0.3
E1, E2 = 60, 80
MAGIC = 0.165745444183859

_NC = None


def _build_nc():
    import concourse.bass as bass
    from concourse import mybir
    from contextlib import ExitStack

    f32 = mybir.dt.float32
    Act = mybir.ActivationFunctionType

    nc = bass.Bass("TRN2", target_bir_lowering=False, debug=False,
                   num_devices=N_CORES)
    x = nc.dram_tensor("x", [R, C], f32, kind="ExternalInput")
    # s0[p, t] = row-sum of exp for local row t*128 + p (tiles 0..30);
    # cols 31..34 are the quarter partial sums of tile 31
    s0_d = nc.dram_tensor("s0", [P, NT_OUT], f32, kind="ExternalOutput")

    xv = x.ap().rearrange("(t p) c -> p t c", p=P)   # [128, 32, 1000]
    HT, LT = NT - 2, NT - 1                          # tiles 30 and 31
    hslot = HT % NBUF                                # tile 30's ring slot
    lslot = NBUF                                     # dedicated slot for 31

    starts = []
    t0 = 0
    for s in CHUNKS:
        starts.append(t0)
        t0 += s
    NCH = len(CHUNKS)
    # chunk c occupies SBUF tile slots [starts[c] % NBUF, ... + size)
    for c, s in enumerate(CHUNKS):
        assert starts[c] % NBUF + s <= NBUF, "chunk wraps SBUF slot ring"

    with ExitStack() as ctx:
        xbuf = ctx.enter_context(nc.sbuf_tensor([P, NBUF + 1, C], f32))
        s0 = ctx.enter_context(nc.sbuf_tensor([P, NT_OUT], f32))

        chunk_sems = [ctx.enter_context(nc.semaphore(f"xc{c}"))
                      for c in range(NCH)]
        h_sems = [ctx.enter_context(nc.semaphore(f"xh{h}"))
                  for h in range(2)]
        q_sems = [ctx.enter_context(nc.semaphore(f"xq{q}"))
                  for q in range(4)]
        act_sem = ctx.enter_context(nc.semaphore("act_sem"))    # tiles exp'd
        dve_sem = ctx.enter_context(nc.semaphore("dve_sem"))    # tiles reduced
        out_sem = ctx.enter_context(nc.semaphore("out_sem"))

        with nc.Block(no_gpsimd_drain=True) as block:

            # DVE op order: tile0, q1-q3, tiles 1..29, h1, h2, q4.
            # consumed_count(tile k) = 1 for k=0, else k+4.
            def consumed(k):
                return 1 if k == 0 else k + 4

            @block.sync
            def _(sync):
                for c, s in enumerate(CHUNKS):
                    t0, slot = starts[c], starts[c] % NBUF
                    if t0 + s > NBUF:
                        sync.wait_ge(dve_sem, consumed(t0 + s - 1 - NBUF))
                    sync.dma_start(xbuf[:, slot:slot + s], xv[:, t0:t0 + s]) \
                        .then_inc(chunk_sems[c], 16)
                    if c == 0:
                        # tile 31 quarters 0-2 as ONE early 375 KB DMA
                        # (three 128 KB transfers sit in the worst DMA
                        # size class); compute still runs per-quarter
                        sync.dma_start(
                            xbuf[:, lslot, 0:3 * CQ],
                            x.ap()[LT * P:(LT + 1) * P, 0:3 * CQ]) \
                            .then_inc(q_sems[0], 16)
                sync.wait_ge(dve_sem, consumed(HT - NBUF))  # free slot 14
                for h in range(2):                          # tile 30 halves
                    sync.dma_start(
                        xbuf[:, hslot, h * CH:(h + 1) * CH],
                        x.ap()[HT * P:(HT + 1) * P, h * CH:(h + 1) * CH]) \
                        .then_inc(h_sems[h], 16)
                sync.dma_start(                             # last 125 KB
                    xbuf[:, lslot, 3 * CQ:],
                    x.ap()[LT * P:(LT + 1) * P, 3 * CQ:]) \
                    .then_inc(q_sems[3], 16)
                # single output DMA: splitting it cannot help — any small
                # final piece pays the same ~2 us issue + completion-sem
                # latency that the whole 18 KB write pays
                sync.wait_ge(dve_sem, NT_OUT)
                sync.dma_start(s0_d.ap(), s0[:]).then_inc(out_sem, 16)
                sync.wait_ge(out_sem, 16)

            @block.scalar
            def _(scalar):
                for c, s in enumerate(CHUNKS):
                    t0, slot = starts[c], starts[c] % NBUF
                    scalar.wait_ge(chunk_sems[c], 16)
                    for j in range(s):
                        scalar.activation(xbuf[:, slot + j], xbuf[:, slot + j],
                                          Act.Exp).then_inc(act_sem)
                    if c == 0:
                        scalar.wait_ge(q_sems[0], 16)
                        for q in range(3):
                            scalar.activation(
                                xbuf[:, lslot, q * CQ:(q + 1) * CQ],
                                xbuf[:, lslot, q * CQ:(q + 1) * CQ],
                                Act.Exp).then_inc(act_sem)
                for h in range(2):
                    scalar.wait_ge(h_sems[h], 16)
                    scalar.activation(xbuf[:, hslot, h * CH:(h + 1) * CH],
                                      xbuf[:, hslot, h * CH:(h + 1) * CH],
                                      Act.Exp).then_inc(act_sem)
                scalar.wait_ge(q_sems[3], 16)
                scalar.activation(xbuf[:, lslot, 3 * CQ:],
                                  xbuf[:, lslot, 3 * CQ:],
                                  Act.Exp).then_inc(act_sem)

            @block.vector
            def _(vector):
                # act_sem counts: tile0=1, q1-q3=2..4, tiles1..29=5..33,
                # h1=34, h2=35, q4=36
                vector.wait_ge(act_sem, 1)
                vector.reduce_sum(s0[:, 0:1], xbuf[:, 0],
                                  axis=mybir.AxisListType.X).then_inc(dve_sem)
                for q in range(3):
                    vector.wait_ge(act_sem, 2 + q)
                    vector.reduce_sum(s0[:, 32 + q:33 + q],
                                      xbuf[:, lslot, q * CQ:(q + 1) * CQ],
                                      axis=mybir.AxisListType.X) \
                        .then_inc(dve_sem)
                for t in range(1, HT):
                    vector.wait_ge(act_sem, t + 4)
                    vector.reduce_sum(s0[:, t:t + 1], xbuf[:, t % NBUF],
                                      axis=mybir.AxisListType.X) \
                        .then_inc(dve_sem)
                for h in range(2):
                    vector.wait_ge(act_sem, 34 + h)
                    vector.reduce_sum(s0[:, 30 + h:31 + h],
                                      xbuf[:, hslot, h * CH:(h + 1) * CH],
                                      axis=mybir.AxisListType.X) \
                        .then_inc(dve_sem)
                vector.wait_ge(act_sem, 36)
                vector.reduce_sum(s0[:, 35:36], xbuf[:, lslot, 3 * CQ:],
                                  axis=mybir.AxisListType.X).then_inc(dve_sem)
    return nc


def _get_nc():
    global _NC
    if _NC is None:
        _NC = _build_nc()
    return _NC


def _margins(cls_num_list, class_difficulty, epoch):
    cls = np.asarray(cls_num_list, dtype=np.float32)
    diff = np.asarray(class_difficulty, dtype=np.float32)
    max_m = np.float32(-np.log(cls.min() / cls.sum()) - np.float32(MAGIC))
    cls_p = (1.0 / np.sqrt(cls)).astype(np.float32)
    m_list = (max_m * cls_p / cls_p.max()).astype(np.float32)
    w = (ALPHA * diff ** POW_P + BETA).astype(np.float32)
    w = (w * (max_m / w.max())).astype(np.float32)
    ep = int(epoch)
    if ep < E1:
        m1 = m_list
    else:
        ee = 1.0 if ep >= E2 else (ep - E1) / (E2 - E1)
        m1 = (m_list + w * (ee / 2)).astype(np.float32)
    return m1


def _in_maps(x, targets, cls_num_list, class_difficulty, epoch):
    x = np.ascontiguousarray(np.asarray(x, dtype=np.float32))
    maps = [{"x": x[cid * R:(cid + 1) * R]} for cid in range(N_CORES)]
    return maps


def run_device(in_maps, trace=False, tmpdir=None):
    from concourse.bass_utils import run_bass_kernel_spmd
    kw = {}
    if trace:
        kw = dict(trace=True, tmpdir=tmpdir, trace_cores=list(range(N_CORES)))
    return run_bass_kernel_spmd(_get_nc(), in_maps,
                                core_ids=list(range(N_CORES)), **kw)


def _host_reference(x, tgt, m1):
    # numerically-stable fallback, never taken for the spec's randn inputs
    z = x.astype(np.float64).copy()
    rows = np.arange(B)
    z[rows, tgt] -= m1[tgt].astype(np.float64)
    mx = z.max(axis=1, keepdims=True)
    lse = np.log(np.exp(z - mx).sum(axis=1)) + mx[:, 0]
    return np.float32((lse - z[rows, tgt]).mean())


def kernel(x, targets, cls_num_list, class_difficulty, epoch):
    x = np.ascontiguousarray(np.asarray(x, dtype=np.float32))
    tgt = np.asarray(targets).astype(np.int64)
    m1 = _margins(cls_num_list, class_difficulty, epoch)
    if not np.isfinite(x).all() or np.abs(x).max() > 70.0:
        # exp without max-subtraction would overflow f32; spec fill is
        # randn so this never triggers in practice
        return _host_reference(x, tgt, m1)
    res = run_device(_in_maps(x, targets, cls_num_list,
                              class_difficulty, epoch))
    # s0[p, t] -> per-row order: row = t*128 + p within each core's shard;
    # the last tile's row-sums arrive as NQ quarter partial sums
    parts = []
    for r in res.results:
        s = r["s0"]                                            # [128, 36]
        parts.append(s[:, :NT - 2].T.reshape(-1))              # tiles 0..29
        parts.append(s[:, 30:32].sum(axis=1))                  # tile 30
        parts.append(s[:, 32:36].sum(axis=1))                  # tile 31
    s0 = np.concatenate(parts)                                 # [B]
    xt = x[np.arange(B), tgt].astype(np.float64)
    m = m1[tgt].astype(np.float64)
    s = s0.astype(np.float64) - np.exp(xt) + np.exp(xt - m)
    loss = np.log(s) - (xt - m)
    return np.float32(loss.mean())



# revision 2
# speedup vs baseline: 1.1321x; 1.1321x over previous
"""CBAM-loss (LDAM-style margin cross-entropy) Trainium2 kernel.

Contract: kernel(**inputs) takes the FULL unsharded inputs
(x [32768, 1000] f32, targets [32768] int, cls_num_list [1000] f32,
class_difficulty [1000] f32, epoch int) and returns the scalar mean
loss (float32), matching:

    m_list1 = margins(cls_num_list, class_difficulty, epoch)   # [C]
    out = x; out[i, t_i] -= m_list1[t_i]
    loss = -mean_i(log_softmax(out)[i, t_i])

Decomposition: per row i with xt_i = x[i, t_i], m_i = m_list1[t_i],

    S0_i   = sum_j exp(x_ij)                       <- device (O(B*C))
    S_i    = S0_i - exp(xt_i) + exp(xt_i - m_i)    <- host (O(B))
    loss_i = log(S_i) - (xt_i - m_i)               <- host (O(B))

x ~ N(0,1), so exp(x) needs no max-subtraction in f32. The device does
the single O(B*C) pass; the O(B) gathers, margin tables ("__init__"
constants) and epilogue stay on the host.

Sharding: data-parallel, 4096 rows per core across 8 NeuronCores.

Device kernel (v2): the memory roofline is ~358 GB/s per core -> 45.8 us
for the 16.4 MB shard. The v1 kernel lost ~10 us to HWDGE descriptor
generation: with the row-interleaved SBUF layout ((t p) c -> p t c) every
4000-byte row needs its own DMA descriptor (4096 per core), and the Sync
sequencer's DIRECT2D instructions became the DMA issue bottleneck
(~30 us serialized), leaving the 16 SDMA engines ~26% idle.

v2 uses the partition-major layout ((p t) c -> p t c): partition p holds
rows p*32..p*32+31 of the shard, a CONTIGUOUS 128,000-byte block of DRAM.
A chunk of any number of row-slots is then 1 descriptor per partition
(128 per chunk, vs 128 per SLOT before). The whole shard fits in SBUF
(125 KiB/partition of ~208), so all chunk DMAs are issued up-front with
no flow control and the stream runs at the HBM limit.

Compute: ScalarE activation(Exp, accum_out=...) computes exp AND the
per-row sum in one instruction (~1.1 us per 1000-col row-slot), so the
Vector engine drops out of the pipeline entirely. Chunk sizes taper
(4,4,4,4,3,3,2,2,2,1,1,1,+2 halves) so ACT, which consumes a chunk only
after its completion semaphore, never lags the stream by more than the
last chunk's compute (~0.6 us). A dummy exp warms the ACT function table
(~1.3 us) during the first chunk's DMA.

Raw Bass (not Tile): every cross-engine wait is a standalone wait_ge.
"""

import numpy as np

B, C = 32768, 1000
N_CORES = 8
R = B // N_CORES          # 4096 rows per core
P = 128                   # SBUF partitions
NT = R // P               # 32 row-slots per partition (contiguous in DRAM)
CH = C // 2               # 500-column half of the last slot
# DMA chunk sizes in row-slots. Tapered: ACT's exp+accum pace is ~1.1 us
# per slot vs DMA's ~1.43 us per slot, but ACT only sees a chunk once its
# last byte lands, so each chunk must not be much larger than ~1.3x the
# next one or ACT backlogs at the stream tail. The final slot arrives as
# two 500-column halves (0.64 us of tail compute instead of 1.3).
CHUNKS = [4, 4, 4, 4, 3, 3, 2, 2, 2, 1, 1, 1]
assert sum(CHUNKS) == NT - 1
NS0 = NT + 1              # 33 s0 columns: slots 0..30, then 2 halves of 31

ALPHA, POW_P, BETA = 0.5, 2.0, 0.3
E1, E2 = 60, 80
MAGIC = 0.165745444183859

_NC = None


def _build_nc():
    import concourse.bass as bass
    from concourse import mybir
    from contextlib import ExitStack

    f32 = mybir.dt.float32
    Act = mybir.ActivationFunctionType

    nc = bass.Bass("TRN2", target_bir_lowering=False, debug=False,
                   num_devices=N_CORES)
    x = nc.dram_tensor("x", [R, C], f32, kind="ExternalInput")
    # s0[p, t] = sum_j exp(x[p*32 + t, j]); cols 31/32 are the two
    # column-halves of slot 31 (host adds them)
    s0_d = nc.dram_tensor("s0", [P, NS0], f32, kind="ExternalOutput")

    # partition p <- rows p*NT .. p*NT+NT-1 (contiguous 128 KB DRAM block)
    xv = x.ap().rearrange("(p t) c -> p t c", p=P)   # [128, 32, 1000]
    LT = NT - 1                                      # slot 31, split in halves

    starts = []
    t0 = 0
    for s in CHUNKS:
        starts.append(t0)
        t0 += s
    NCH = len(CHUNKS)
    N_ACT = NT + 1            # 31 full-slot activations + 2 halves

    with ExitStack() as ctx:
        xbuf = ctx.enter_context(nc.sbuf_tensor([P, NT, C], f32))
        s0 = ctx.enter_context(nc.sbuf_tensor([P, NS0], f32))
        warm = ctx.enter_context(nc.sbuf_tensor([P, 1], f32))

        chunk_sems = [ctx.enter_context(nc.semaphore(f"xc{c}"))
                      for c in range(NCH)]
        h_sems = [ctx.enter_context(nc.semaphore(f"xh{h}"))
                  for h in range(2)]
        act_sem = ctx.enter_context(nc.semaphore("act_sem"))
        out_sem = ctx.enter_context(nc.semaphore("out_sem"))

        with nc.Block(no_gpsimd_drain=True) as block:

            @block.sync
            def _(sync):
                # whole shard is SBUF-resident: issue every chunk DMA
                # up-front, back-to-back; the HWDGE ring drains them at
                # the HBM rate with no inter-chunk dependency
                for c, s in enumerate(CHUNKS):
                    t0 = starts[c]
                    sync.dma_start(xbuf[:, t0:t0 + s], xv[:, t0:t0 + s]) \
                        .then_inc(chunk_sems[c], 16)
                for h in range(2):
                    sync.dma_start(
                        xbuf[:, LT, h * CH:(h + 1) * CH],
                        xv[:, LT, h * CH:(h + 1) * CH]) \
                        .then_inc(h_sems[h], 16)
                sync.wait_ge(act_sem, N_ACT)
                sync.dma_start(s0_d.ap(), s0[:]).then_inc(out_sem, 16)
                sync.wait_ge(out_sem, 16)

            @block.scalar
            def _(scalar):
                # dummy exp: loads the ACT exp table (~1.3 us) while the
                # first chunk is still in flight
                scalar.activation(warm[:], warm[:], Act.Exp)
                for c, s in enumerate(CHUNKS):
                    t0 = starts[c]
                    scalar.wait_ge(chunk_sems[c], 16)
                    for j in range(s):
                        t = t0 + j
                        scalar.activation(xbuf[:, t], xbuf[:, t], Act.Exp,
                                          accum_out=s0[:, t:t + 1]) \
                            .then_inc(act_sem)
                for h in range(2):
                    scalar.wait_ge(h_sems[h], 16)
                    scalar.activation(xbuf[:, LT, h * CH:(h + 1) * CH],
                                      xbuf[:, LT, h * CH:(h + 1) * CH],
                                      Act.Exp,
                                      accum_out=s0[:, NT - 1 + h:NT + h]) \
                        .then_inc(act_sem)
    return nc


def _get_nc():
    global _NC
    if _NC is None:
        _NC = _build_nc()
    return _NC


def _margins(cls_num_list, class_difficulty, epoch):
    cls = np.asarray(cls_num_list, dtype=np.float32)
    diff = np.asarray(class_difficulty, dtype=np.float32)
    max_m = np.float32(-np.log(cls.min() / cls.sum()) - np.float32(MAGIC))
    cls_p = (1.0 / np.sqrt(cls)).astype(np.float32)
    m_list = (max_m * cls_p / cls_p.max()).astype(np.float32)
    w = (ALPHA * diff ** POW_P + BETA).astype(np.float32)
    w = (w * (max_m / w.max())).astype(np.float32)
    ep = int(epoch)
    if ep < E1:
        m1 = m_list
    else:
        ee = 1.0 if ep >= E2 else (ep - E1) / (E2 - E1)
        m1 = (m_list + w * (ee / 2)).astype(np.float32)
    return m1


def _in_maps(x, targets, cls_num_list, class_difficulty, epoch):
    x = np.ascontiguousarray(np.asarray(x, dtype=np.float32))
    maps = [{"x": x[cid * R:(cid + 1) * R]} for cid in range(N_CORES)]
    return maps


def run_device(in_maps, trace=False, tmpdir=None):
    from concourse.bass_utils import run_bass_kernel_spmd
    kw = {}
    if trace:
        kw = dict(trace=True, tmpdir=tmpdir, trace_cores=list(range(N_CORES)))
    return run_bass_kernel_spmd(_get_nc(), in_maps,
                                core_ids=list(range(N_CORES)), **kw)


def _host_reference(x, tgt, m1):
    # numerically-stable fallback, never taken for the spec's randn inputs
    z = x.astype(np.float64).copy()
    rows = np.arange(B)
    z[rows, tgt] -= m1[tgt].astype(np.float64)
    mx = z.max(axis=1, keepdims=True)
    lse = np.log(np.exp(z - mx).sum(axis=1)) + mx[:, 0]
    return np.float32((lse - z[rows, tgt]).mean())


def kernel(x, targets, cls_num_list, class_difficulty, epoch):
    x = np.ascontiguousarray(np.asarray(x, dtype=np.float32))
    tgt = np.asarray(targets).astype(np.int64)
    m1 = _margins(cls_num_list, class_difficulty, epoch)
    if not np.isfinite(x).all() or np.abs(x).max() > 70.0:
        # exp without max-subtraction would overflow f32; spec fill is
        # randn so this never triggers in practice
        return _host_reference(x, tgt, m1)
    res = run_device(_in_maps(x, targets, cls_num_list,
                              class_difficulty, epoch))
    # s0[p, t] -> shard row p*32 + t; cols 31/32 are slot 31's halves
    parts = []
    for r in res.results:
        s = r["s0"]                                            # [128, 33]
        rows = np.concatenate(
            [s[:, :NT - 1], (s[:, NT - 1] + s[:, NT])[:, None]], axis=1)
        parts.append(rows.reshape(-1))                         # [4096]
    s0 = np.concatenate(parts)                                 # [B]
    xt = x[np.arange(B), tgt].astype(np.float64)
    m = m1[tgt].astype(np.float64)
    s = s0.astype(np.float64) - np.exp(xt) + np.exp(xt - m)
    loss = np.log(s) - (xt - m)
    return np.float32(loss.mean())
